# revision 1
# baseline (speedup 1.0000x reference)
"""Trainium2 Bass kernel for nn_CrossAttentionLayer (sparse cross attention).

Sharding: 8 cores = 4 batches x 2 head-groups. Core c handles batch c//2 and
heads [4*(c%2), 4*(c%2)+4). Each core computes LN + q/k/v projections for its
shard, flash-style masked attention in transposed layout, and a partial
out-projection. Host sums the two per-batch partials and adds bo.

Device algorithm (per core), all matmuls bf16 with fp32 PSUM accumulation:
  xlnT   = transpose(layernorm(x))            (LN gains/biases folded into W/b)
  qT/kT  = W.T @ xlnT   [d, tok]              (per-partition bias via ACT)
  v      = xlnT.T @ Wv  [tok, d]  * kv_mask   (kv_mask folded into v + ones col)
  sT     = kT.T-blocks @ qT-blocks            [k, q] scores, transposed
  pT     = exp(sT * scale) * sparse_mask.T    (ACT exp + DVE mask multiply)
  accT   = [v | kvm].T @ pT                   rows 0-63 = unnormalized out.T,
                                              row 64 = softmax denominator
  aT     = accT[0:64] * (1/denominator)       (broadcast via DMA replicate)
  out    = aT.T-blocks @ Wo-blocks            [q, E] partial, fp32 to HBM
"""

import os

import numpy as np
import ml_dtypes

import bass_rust
import concourse.bass as bass
import concourse.mybir as mybir
import concourse.tile as tile
from concourse import bass_utils
from concourse.masks import make_identity
from concourse.vector_clock import ScopedClock


class _TileContext(tile.TileContext):
    """TileContext whose kernel-tail drain is split into single-wait drains.

    The walrus build in this environment rejects >1 sync-wait on a Drain
    (CTRL_NO struct): "Too many sync wait commands". The stock
    _drain_and_barrier attaches one wait per outstanding semaphore to a
    single Drain; emit one Drain per wait instead.
    """

    def _drain_and_barrier(self, tick_clock, wait_clock):
        drain_inst = self.nc.sync.drain()
        wait_clock.add_sem_waits(
            drain_inst.ins, ScopedClock({None: tick_clock.global_clock})
        )
        si = drain_inst.ins.sync_info
        if si is not None and si.on_wait and len(si.on_wait) > 1:
            waits = list(si.on_wait)
            drain_inst.ins.sync_info = bass_rust.SyncInfo(
                on_wait=[waits[0]], on_update=si.on_update or [])
            for w in waits[1:]:
                extra = self.nc.sync.drain()
                extra.ins.sync_info = bass_rust.SyncInfo(
                    on_wait=[w], on_update=[])

        self.nc.all_engine_barrier()
        assert self.sems is not None
        popped = self.nc._tile_sem_poison_stack.pop()
        assert popped is self._sem_poison
        self.nc.clear_and_free_semaphores(list(self.sems.allocated().values()))
        self.nc.all_engine_barrier()

def _split_sync_waits(nc):
    """Cap every instruction at one sync wait.

    This walrus build rejects instructions carrying more than one sem wait
    ("Too many sync wait commands", setupSyncWait) across several structs
    (Drain, DMACopy, ...). Move excess waits onto no-op instructions placed
    immediately before the offender on the same engine — identical ordering
    semantics, one wait per instruction.
    """
    for f in nc.m.functions:
        for bb in f.blocks:
            insns = bb.instructions
            out = []
            changed = False
            for ins in insns:
                si = ins.sync_info
                if si is not None and si.on_wait and len(si.on_wait) > 1:
                    waits = list(si.on_wait)
                    for w in waits[:-1]:
                        nop = mybir.InstNoOp(
                            name=nc.get_next_instruction_name(),
                            engine=ins.engine,
                            ins=[], outs=[],
                            sync_info=bass_rust.SyncInfo(
                                on_wait=[w], on_update=[]),
                        )
                        out.append(nop)
                    ins.sync_info = bass_rust.SyncInfo(
                        on_wait=[waits[-1]], on_update=si.on_update or [])
                    changed = True
                out.append(ins)
            if changed:
                bb.instructions = out


BF16 = ml_dtypes.bfloat16

E = 512
H = 8
D = 64
T = 2048           # tokens (both query and key side)
P = 128
NT = T // P        # 16 token tiles
EC = E // P        # 4 contraction chunks
HC = 4             # heads per core
MC = 2             # 128-wide chunks of this core's 256 head dims
QC = 4             # 512-wide query chunks
SCALE = float(D) ** -0.5
EPS = 1e-5

_CACHE = {}


def _build(needs_bv: bool, reps: int = 1):
    nc = bass.Bass("TRN2", target_bir_lowering=False, debug=False, num_devices=8)
    f32 = mybir.dt.float32
    bf16 = mybir.dt.bfloat16

    xq = nc.dram_tensor("xq", [T, E], f32, kind="ExternalInput").ap()
    xkv = nc.dram_tensor("xkv", [T, E], f32, kind="ExternalInput").ap()
    wq = nc.dram_tensor("wq", [E, MC * P], bf16, kind="ExternalInput").ap()
    wk = nc.dram_tensor("wk", [E, MC * P], bf16, kind="ExternalInput").ap()
    wv = nc.dram_tensor("wv", [E, MC * P], bf16, kind="ExternalInput").ap()
    wo = nc.dram_tensor("wo", [MC * P, E], bf16, kind="ExternalInput").ap()
    bqd = nc.dram_tensor("bq", [P, MC], f32, kind="ExternalInput").ap()
    bkd = nc.dram_tensor("bk", [P, MC], f32, kind="ExternalInput").ap()
    kvmd = nc.dram_tensor("kvm", [P, NT], f32, kind="ExternalInput").ap()
    mtd = nc.dram_tensor("mt", [T, T], bf16, kind="ExternalInput").ap()
    if needs_bv:
        bvd = nc.dram_tensor("bv", [1, MC * P], bf16, kind="ExternalInput").ap()
    outd = nc.dram_tensor("out", [T, E], f32, kind="ExternalOutput").ap()

    with _TileContext(nc) as tc:
        with (
            tc.tile_pool(name="persist", bufs=1) as pp,
            tc.tile_pool(name="xs", bufs=5) as xpool,
            tc.tile_pool(name="work", bufs=5) as wk_pool,
            tc.tile_pool(name="scratch", bufs=4) as scratch,
            tc.tile_pool(name="psA", bufs=2, space="PSUM") as psA,
            tc.tile_pool(name="psS", bufs=2, space="PSUM") as psS,
            tc.tile_pool(name="psC", bufs=2, space="PSUM") as psC,
        ):
            # ---- persistent SBUF tensors ----
            # xlnT / qT / aT are split into per-group tensors so downstream
            # consumers unblock as soon as their group is written (Tile tracks
            # dependencies per tile, so monolithic tensors serialize phases).
            wq_sb = pp.tile([P, EC, MC * P], bf16, tag="wq")
            wk_sb = pp.tile([P, EC, MC * P], bf16, tag="wk")
            wv_sb = pp.tile([P, EC, MC * P], bf16, tag="wv")
            wo_sb = pp.tile([P, MC, E], bf16, tag="wo")
            bq_sb = pp.tile([P, MC], f32, tag="bq")
            bk_sb = pp.tile([P, MC], f32, tag="bk")
            kvm_sb = pp.tile([P, NT], f32, tag="kvm")
            mt_gt = [pp.tile([P, 4, T], bf16, tag=f"mt{g}", name=f"mt{g}")
                     for g in range(4)]
            xlnq_g = [pp.tile([P, 4, EC, P], bf16, tag=f"xlnq{g}",
                              name=f"xlnq{g}") for g in range(4)]
            xlnkv_g = [pp.tile([P, 4, EC, P], bf16, tag=f"xlnkv{g}",
                               name=f"xlnkv{g}") for g in range(4)]
            qT_g = [pp.tile([P, MC, 512], bf16, tag=f"qT{g}", name=f"qT{g}")
                    for g in range(4)]
            kT_gt = [pp.tile([P, MC, 512], bf16, tag=f"kT{g}", name=f"kT{g}")
                     for g in range(4)]
            v_gt = [pp.tile([P, 4, HC * (D + 1)], bf16, tag=f"v{g}",
                            name=f"v{g}") for g in range(4)]
            aT_g = [pp.tile([P, MC, 512], bf16, tag=f"aT{g}", name=f"aT{g}")
                    for g in range(4)]
            sums_g = [pp.tile([P, HC * 4], f32, tag=f"sums{g}",
                              name=f"sums{g}") for g in range(QC)]
            rsp_g = [pp.tile([P, HC * 4], bf16, tag=f"rsp{g}",
                             name=f"rsp{g}") for g in range(QC)]
            rs_flat_g = [pp.tile([1, HC, 512], bf16, tag=f"rsflat{g}",
                                 name=f"rsflat{g}") for g in range(QC)]
            if needs_bv:
                bv_sb = pp.tile([1, MC * P], bf16, tag="bv")
                ones_sb = pp.tile([1, P], bf16, tag="ones")

            eps_sb = pp.tile([P, 1], f32, tag="eps")
            nc.vector.memset(eps_sb[:], EPS)
            if needs_bv:
                nc.sync.dma_start(bv_sb[:], bvd)
                nc.vector.memset(ones_sb[:], 1.0)
            ident = pp.tile([P, P], bf16, tag="ident")
            make_identity(nc, ident[:])
            ones1 = pp.tile([1, D], bf16, tag="ones1")
            nc.vector.memset(ones1[:], 1.0)
            mtr = mtd.rearrange("(c p) q -> p c q", p=P)

            def ln_group(src, dstT, g):
                """LN 4 token tiles of src into dstT ([P, 4, EC, P])."""
                for tp in range(2):
                    # one [128, 1024] bf16 psum tile (1 bank) holds the
                    # transposes of a PAIR of token tiles; one wide copy out
                    ptr = psA.tile([P, 2, E], bf16, tag="p512")
                    for ti2 in range(2):
                        ti = tp * 2 + ti2
                        t = g * 4 + ti
                        xt = xpool.tile([P, E], f32, tag="x")
                        nc.sync.dma_start(xt[:], src[t * P:(t + 1) * P, :])
                        stats = scratch.tile([P, 6], f32, tag="bnstats")
                        mv = scratch.tile([P, 2], f32, tag="bnmv")
                        nc.vector.bn_stats(stats[:], xt[:])
                        nc.vector.bn_aggr(mv[:], stats[:])
                        sig = scratch.tile([P, 1], f32, tag="sig")
                        nc.scalar.activation(
                            sig[:], mv[:, 1:2],
                            mybir.ActivationFunctionType.Sqrt,
                            bias=eps_sb[:])
                        rsig = scratch.tile([P, 1], f32, tag="rsig")
                        nc.vector.reciprocal(rsig[:], sig[:])
                        xln = wk_pool.tile([P, E], bf16, tag="xln")
                        nc.vector.tensor_scalar(
                            xln[:], xt[:], mv[:, 0:1], rsig[:],
                            mybir.AluOpType.subtract, mybir.AluOpType.mult)
                        for c in range(EC):
                            nc.tensor.transpose(
                                ptr[:, ti2, c * P:(c + 1) * P],
                                xln[:, c * P:(c + 1) * P], ident[:])
                    nc.vector.tensor_copy(
                        dstT[:, 2 * tp:2 * tp + 2],
                        ptr[:].rearrange("p u (c n) -> p u c n", n=P))

            def kproj_group(g):
                for mc in range(MC):
                    ps = psA.tile([P, 512], mybir.dt.float32, tag="p512")
                    for c in range(EC):
                        nc.tensor.matmul(
                            ps[:],
                            lhsT=wk_sb[:, c, mc * P:(mc + 1) * P],
                            rhs=xlnkv_g[g][:, :, c, :],
                            start=(c == 0), stop=(c == EC - 1))
                    nc.scalar.activation(
                        kT_gt[g][:, mc, :], ps[:],
                        mybir.ActivationFunctionType.Identity,
                        bias=bk_sb[:, mc:mc + 1])

            def vproj_group(g):
                for ti in range(4):
                    t = g * 4 + ti
                    ps = psA.tile([P, MC * P], mybir.dt.float32, tag="p512")
                    for c in range(EC):
                        nc.tensor.matmul(
                            ps[:],
                            lhsT=xlnkv_g[g][:, ti, c, :],
                            rhs=wv_sb[:, c, :],
                            start=(c == 0),
                            stop=(c == EC - 1 and not needs_bv))
                    if needs_bv:
                        nc.tensor.matmul(
                            ps[:], lhsT=ones_sb[:], rhs=bv_sb[:],
                            start=False, stop=True)
                    vd = v_gt[g][:, ti].rearrange("p (h d) -> p h d", d=D + 1)
                    nc.vector.tensor_scalar(
                        vd[:, :, 0:D], ps.rearrange("p (h d) -> p h d", d=D),
                        kvm_sb[:, t:t + 1], None, mybir.AluOpType.mult)
                    nc.vector.tensor_copy(
                        vd[:, :, D], kvm_sb[:, t:t + 1].to_broadcast((P, HC)))

            def qproj_group(g):
                for mc in range(MC):
                    ps = psA.tile([P, 512], mybir.dt.float32, tag="p512")
                    for c in range(EC):
                        nc.tensor.matmul(
                            ps[:],
                            lhsT=wq_sb[:, c, mc * P:(mc + 1) * P],
                            rhs=xlnq_g[g][:, :, c, :],
                            start=(c == 0), stop=(c == EC - 1))
                    nc.scalar.activation(
                        qT_g[g][:, mc, :], ps[:],
                        mybir.ActivationFunctionType.Identity,
                        bias=bq_sb[:, mc:mc + 1])

            # kv side first (attention needs all of kT/v); mask chunks stream
            # in behind the x loads; q-side groups unblock attention per qc.
            rep_ctx = tc.For_i(0, reps, 1) if reps > 1 else None
            if rep_ctx is not None:
                rep_ctx.__enter__()
            for g in range(4):
                ln_group(xkv, xlnkv_g[g], g)
                if g == 0:
                    # weights land behind the first x tiles on the DMA queues
                    nc.sync.dma_start(
                        wk_sb[:], wk.rearrange("(c p) n -> p c n", p=P))
                    nc.sync.dma_start(
                        wv_sb[:], wv.rearrange("(c p) n -> p c n", p=P))
                    nc.sync.dma_start(
                        wq_sb[:], wq.rearrange("(c p) n -> p c n", p=P))
                    nc.sync.dma_start(
                        wo_sb[:], wo.rearrange("(c p) n -> p c n", p=P))
                    nc.sync.dma_start(bq_sb[:], bqd)
                    nc.sync.dma_start(bk_sb[:], bkd)
                    nc.sync.dma_start(kvm_sb[:], kvmd)
                kproj_group(g)
                vproj_group(g)
                for c in range(4):
                    nc.sync.dma_start(mt_gt[g][:, c], mtr[:, 4 * g + c])
                if g == 1:
                    # q group 0 early: attention (qc=0) starts on k groups
                    # 0-1 while kv groups 2-3 are still in layernorm
                    ln_group(xq, xlnq_g[0], 0)
                    qproj_group(0)
            for g in range(1, 4):
                ln_group(xq, xlnq_g[g], g)
                qproj_group(g)

            # ---- attention: qc outer so normalize+out_proj overlap ----
            for qc in range(QC):
                for h in range(HC):
                    mc = h // 2
                    po = (h % 2) * D
                    acc = psC.tile([P, 512], mybir.dt.float32, tag="acc")
                    for kcp in range(NT // 2):
                        sp = psS.tile([P, 2, 512], mybir.dt.float32, tag="sp")
                        kg = kcp // 2          # k group (4 k-chunks each)
                        ko = (2 * kcp) % 4     # chunk offset inside group
                        for j in range(2):
                            nc.tensor.matmul(
                                sp[:, j],
                                lhsT=kT_gt[kg][po:po + D, mc,
                                               (ko + j) * P:(ko + j + 1) * P],
                                rhs=qT_g[qc][po:po + D, mc, :],
                                start=True, stop=True)
                        pT = wk_pool.tile([P, 2, 512], bf16, tag="pT")
                        nc.scalar.activation(
                            pT[:], sp[:], mybir.ActivationFunctionType.Exp,
                            scale=SCALE)
                        nc.vector.tensor_tensor(
                            pT[:], pT[:],
                            mt_gt[kg][:, ko:ko + 2, qc * 512:(qc + 1) * 512],
                            mybir.AluOpType.mult)
                        for j in range(2):
                            nc.tensor.matmul(
                                acc[:D + 1],
                                lhsT=v_gt[kg][:, ko + j,
                                              h * (D + 1):(h + 1) * (D + 1)],
                                rhs=pT[:, j],
                                start=(kcp == 0 and j == 0),
                                stop=(kcp == NT // 2 - 1 and j == 1))
                    stage = wk_pool.tile([P, 512], f32, tag="sumstage")
                    nc.vector.tensor_copy(stage[D:D + 1, :], acc[D:D + 1, :])
                    # gather this head's denominators into [P, 4] of sums_g
                    # (DMA streams element-linearly: q index = p*4 + f)
                    nc.sync.dma_start(
                        sums_g[qc][:, h * 4:(h + 1) * 4], stage[D:D + 1, :])
                    nc.vector.tensor_copy(
                        aT_g[qc][po:po + D, mc, :], acc[:D])

                    if h % 2 == 1:
                        # this head pair (chunk mc) is complete: normalize it
                        # now so the chain overlaps the remaining heads
                        sl = slice(8 * mc, 8 * mc + 8)
                        nc.vector.tensor_scalar(
                            sums_g[qc][:, sl], sums_g[qc][:, sl], 1e-30, None,
                            mybir.AluOpType.add)
                        rsp = rsp_g[qc]
                        rsf = scratch.tile([P, 8], f32, tag="rsf")
                        nc.vector.reciprocal(rsf[:], sums_g[qc][:, sl])
                        nc.vector.tensor_copy(rsp[:, sl], rsf[:])
                        for hh in (2 * mc, 2 * mc + 1):
                            nc.sync.dma_start(
                                rs_flat_g[qc][:, hh, :],
                                rsp[:, hh * 4:(hh + 1) * 4])
                        rb = psA.tile([P, 512], mybir.dt.float32, tag="p512")
                        nc.tensor.matmul(
                            rb[0:D], lhsT=ones1[:],
                            rhs=rs_flat_g[qc][:, 2 * mc, :],
                            start=True, stop=True)
                        nc.tensor.matmul(
                            rb[D:2 * D], lhsT=ones1[:],
                            rhs=rs_flat_g[qc][:, 2 * mc + 1, :],
                            start=True, stop=True)
                        nc.vector.tensor_tensor(
                            aT_g[qc][:, mc, :], aT_g[qc][:, mc, :],
                            rb[:], mybir.AluOpType.mult)

                # out projection for this qc's 4 token tiles
                for ti in range(4):
                    t = qc * 4 + ti
                    ps = psA.tile([P, E], mybir.dt.float32, tag="p512")
                    for mc in range(MC):
                        nc.tensor.matmul(
                            ps[:],
                            lhsT=aT_g[qc][:, mc, ti * P:(ti + 1) * P],
                            rhs=wo_sb[:, mc, :],
                            start=(mc == 0), stop=(mc == MC - 1))
                    osb = wk_pool.tile([P, E], f32, tag="osb")
                    nc.vector.tensor_copy(osb[:], ps[:])
                    nc.sync.dma_start(outd[t * P:(t + 1) * P, :], osb[:])

            if rep_ctx is not None:
                rep_ctx.__exit__(None, None, None)

    _split_sync_waits(nc)
    return nc


def _get_nc(needs_bv: bool, reps: int = 1):
    key = ("nc", needs_bv, reps)
    if key not in _CACHE:
        _CACHE[key] = _build(needs_bv, reps)
    return _CACHE[key]


def kernel(query, key_value, kv_mask, sparse_mask,
           ln_q_g, ln_q_b, ln_kv_g, ln_kv_b,
           Wq, bq, Wk, bk, Wv, bv, Wo, bo):
    query = np.asarray(query, np.float32)
    key_value = np.asarray(key_value, np.float32)
    kv_mask = np.asarray(kv_mask)
    sparse_mask = np.asarray(sparse_mask)
    B = query.shape[0]

    # Fold LN gain/bias into the projection weights (exact algebra):
    # (x_ln*g + b) @ W + c  ==  x_ln @ (g[:,None]*W) + (b@W + c)
    Wq_g = np.asarray(ln_q_g, np.float32)[:, None] * np.asarray(Wq, np.float32)
    Wk_g = np.asarray(ln_kv_g, np.float32)[:, None] * np.asarray(Wk, np.float32)
    Wv_g = np.asarray(ln_kv_g, np.float32)[:, None] * np.asarray(Wv, np.float32)
    bq_e = np.asarray(ln_q_b, np.float32) @ np.asarray(Wq, np.float32) + bq
    bk_e = np.asarray(ln_kv_b, np.float32) @ np.asarray(Wk, np.float32) + bk
    bv_e = np.asarray(ln_kv_b, np.float32) @ np.asarray(Wv, np.float32) + bv

    needs_bv = bool(np.any(bv_e != 0.0))
    reps = int(os.environ.get("KERNEL_REPS", "1"))
    nc = _get_nc(needs_bv, reps)

    in_maps = []
    for c in range(8):
        b, hg = c // 2, c % 2
        hs = slice(hg * MC * P, (hg + 1) * MC * P)
        m = {
            "xq": np.ascontiguousarray(query[b]),
            "xkv": np.ascontiguousarray(key_value[b]),
            "wq": np.ascontiguousarray(Wq_g[:, hs]).astype(BF16),
            "wk": np.ascontiguousarray(Wk_g[:, hs]).astype(BF16),
            "wv": np.ascontiguousarray(Wv_g[:, hs]).astype(BF16),
            "wo": np.ascontiguousarray(np.asarray(Wo, np.float32)[hs, :]).astype(BF16),
            "bq": np.ascontiguousarray(bq_e[hs].reshape(MC, P).T),
            "bk": np.ascontiguousarray(bk_e[hs].reshape(MC, P).T),
            "kvm": np.ascontiguousarray(
                kv_mask[b].astype(np.float32).reshape(NT, P).T),
            "mt": np.ascontiguousarray(sparse_mask[b].T).astype(BF16),
        }
        if needs_bv:
            m["bv"] = bv_e[hs].astype(BF16).reshape(1, MC * P)
        in_maps.append(m)

    res = bass_utils.run_bass_kernel_spmd(
        nc, in_maps, core_ids=list(range(8)),
        trace=bool(os.environ.get("KERNEL_TRACE")))
    globals()["LAST_RESULTS"] = res

    bo_f = np.asarray(bo, np.float32)
    out = np.empty((B, T, E), np.float32)
    for b in range(B):
        out[b] = res.results[2 * b]["out"] + res.results[2 * b + 1]["out"] + bo_f
    return out



# revision 7
# speedup vs baseline: 1.4587x; 1.4587x over previous
"""Trainium2 Bass kernel for nn_CrossAttentionLayer (sparse cross attention).

Sharding: 8 cores = 4 batches x 2 head-groups. Core c handles batch c//2 and
heads [4*(c%2), 4*(c%2)+4). Host compacts the kv sequence per batch using
kv_mask (a kv token masked out by kv_mask is invisible to every query), padding
to a multiple of 384 tokens; the padded sparse-mask columns are zero so padding
contributes nothing. Each core computes LN + q/k/v projections for its shard,
masked attention, and a partial out-projection. Host sums the two per-batch
partials and adds bo.

Device algorithm (per core), matmuls bf16 with fp32 PSUM accumulation:
  xlnT   = transpose(layernorm(x))          (LN gains/biases folded into W/b)
  qT/kT  = W.T @ xlnT   [d, tok]
  v      = xlnT.T @ Wv  [tok, d] with a ones column per head (softmax denom)
  per (q-block of 512, head):
    sT   = kT-chunks.T @ qT               [k, q] scores, 3-kv-tile chunks
    pT   = exp(sT * scale) * sparse_mask  (ACT exp; mask multiply split
                                           DVE/GPSIMD, GPSIMD chunk last)
    acc  = pT-chunks.T @ [v | 1]          [q, 65] PSUM accumulation over k
    av   = acc[:, 0:64] / acc[:, 64]      (per-partition normalize on DVE)
    aT   = transpose(av)                  (PE transpose, bf16 PSUM staging)
  out    = aT-chunks @ Wo                 [q, E] partial, bf16 to HBM
"""

import math
import os

import numpy as np
import ml_dtypes

import bass_rust
import concourse.bass as bass
import concourse.mybir as mybir
import concourse.tile as tile
from concourse import bass_utils
from concourse.masks import make_identity
from concourse.vector_clock import ScopedClock


class _TileContext(tile.TileContext):
    """TileContext whose kernel-tail drain is split into single-wait drains.

    The walrus build in this environment rejects >1 sync-wait on a Drain
    (CTRL_NO struct): "Too many sync wait commands". The stock
    _drain_and_barrier attaches one wait per outstanding semaphore to a
    single Drain; emit one Drain per wait instead.
    """

    def _drain_and_barrier(self, tick_clock, wait_clock):
        drain_inst = self.nc.sync.drain()
        wait_clock.add_sem_waits(
            drain_inst.ins, ScopedClock({None: tick_clock.global_clock})
        )
        si = drain_inst.ins.sync_info
        if si is not None and si.on_wait and len(si.on_wait) > 1:
            waits = list(si.on_wait)
            drain_inst.ins.sync_info = bass_rust.SyncInfo(
                on_wait=[waits[0]], on_update=si.on_update or [])
            for w in waits[1:]:
                extra = self.nc.sync.drain()
                extra.ins.sync_info = bass_rust.SyncInfo(
                    on_wait=[w], on_update=[])

        self.nc.all_engine_barrier()
        assert self.sems is not None
        popped = self.nc._tile_sem_poison_stack.pop()
        assert popped is self._sem_poison
        self.nc.clear_and_free_semaphores(list(self.sems.allocated().values()))
        self.nc.all_engine_barrier()


def _split_sync_waits(nc):
    """Cap every instruction at one sync wait.

    This walrus build rejects instructions carrying more than one sem wait
    ("Too many sync wait commands", setupSyncWait) across several structs
    (Drain, DMACopy, ...). Move excess waits onto no-op instructions placed
    immediately before the offender on the same engine — identical ordering
    semantics, one wait per instruction.
    """
    for f in nc.m.functions:
        for bb in f.blocks:
            insns = bb.instructions
            out = []
            changed = False
            for ins in insns:
                si = ins.sync_info
                if si is not None and si.on_wait and len(si.on_wait) > 1:
                    waits = list(si.on_wait)
                    for w in waits[:-1]:
                        nop = mybir.InstNoOp(
                            name=nc.get_next_instruction_name(),
                            engine=ins.engine,
                            ins=[], outs=[],
                            sync_info=bass_rust.SyncInfo(
                                on_wait=[w], on_update=[]),
                        )
                        out.append(nop)
                    ins.sync_info = bass_rust.SyncInfo(
                        on_wait=[waits[-1]], on_update=si.on_update or [])
                    changed = True
                out.append(ins)
            if changed:
                bb.instructions = out


BF16 = ml_dtypes.bfloat16

E = 512
H = 8
D = 64
T = 2048           # query tokens
P = 128
NQT = T // P       # 16 query token tiles
EC = E // P        # 4 contraction chunks
HC = 4             # heads per core
MC = 2             # 128-wide chunks of this core's 256 head dims
QC = 4             # 512-wide query chunks (= query tile groups of 4)
SCALE = float(D) ** -0.5
EPS = 1e-5

_CACHE = {}
_LAST_KEY = None


def _build(nkt: int, needs_bqk: bool, needs_bv: bool):
    assert nkt % 3 == 0, "kv tiles padded to a multiple of 3"
    nkg = nkt // 3               # kv groups of 3 tiles / score chunks of 3
    KT = nkt * P

    nc = bass.Bass("TRN2", target_bir_lowering=False, debug=False,
                   num_devices=8)
    f32 = mybir.dt.float32
    bf16 = mybir.dt.bfloat16

    xq = nc.dram_tensor("xq", [T, E], f32, kind="ExternalInput").ap()
    xkv = nc.dram_tensor("xkv", [KT, E], f32, kind="ExternalInput").ap()
    wq = nc.dram_tensor("wq", [E, MC * P], bf16, kind="ExternalInput").ap()
    wk = nc.dram_tensor("wk", [E, MC * P], bf16, kind="ExternalInput").ap()
    wv = nc.dram_tensor("wv", [E, MC * P], bf16, kind="ExternalInput").ap()
    wo = nc.dram_tensor("wo", [MC * P, E], bf16, kind="ExternalInput").ap()
    mtd = nc.dram_tensor("mt", [KT, T], bf16, kind="ExternalInput").ap()
    if needs_bqk:
        bqd = nc.dram_tensor("bq", [P, MC], f32, kind="ExternalInput").ap()
        bkd = nc.dram_tensor("bk", [P, MC], f32, kind="ExternalInput").ap()
    if needs_bv:
        bvd = nc.dram_tensor("bv", [1, MC * P], bf16, kind="ExternalInput").ap()
    outd = nc.dram_tensor("out", [T, E], bf16, kind="ExternalOutput").ap()

    mtr = mtd.rearrange("(c p) q -> p c q", p=P)

    with _TileContext(nc) as tc:
        with (
            tc.tile_pool(name="persist", bufs=1) as pp,
            tc.tile_pool(name="xs", bufs=4) as xpool,
            tc.tile_pool(name="xln", bufs=3) as xlnp,
            tc.tile_pool(name="scratch", bufs=4) as scr,
            tc.tile_pool(name="ostage", bufs=2) as outp,
            tc.tile_pool(name="pt", bufs=2) as pTp,
            tc.tile_pool(name="av", bufs=2) as avp,
            tc.tile_pool(name="at", bufs=2) as aTp,
            tc.tile_pool(name="psS", bufs=2, space="PSUM") as psS,
            tc.tile_pool(name="psA", bufs=1, space="PSUM") as psA,
            tc.tile_pool(name="psT", bufs=1, space="PSUM") as psT,
        ):
            # ---- persistent SBUF tensors ----
            wq_sb = pp.tile([P, EC, MC * P], bf16, tag="wq")
            wk_sb = pp.tile([P, EC, MC * P], bf16, tag="wk")
            wv_sb = pp.tile([P, EC, MC * P], bf16, tag="wv")
            wo_sb = pp.tile([P, MC, E], bf16, tag="wo")
            mt_g = [pp.tile([P, 3, T], bf16, tag=f"mt{i}", name=f"mt{i}")
                    for i in range(nkg)]
            xlnkvT_g = [pp.tile([P, 3, EC, P], bf16, tag=f"xlnkv{i}",
                                name=f"xlnkv{i}") for i in range(nkg)]
            xlnqT_g = [pp.tile([P, 4, EC, P], bf16, tag=f"xlnq{g}",
                               name=f"xlnq{g}") for g in range(QC)]
            kT_g = [pp.tile([P, MC, 3 * P], bf16, tag=f"kT{i}",
                            name=f"kT{i}") for i in range(nkg)]
            v_g = [pp.tile([P, 3, HC * (D + 1)], bf16, tag=f"v{i}",
                           name=f"v{i}") for i in range(nkg)]
            qT_g = [pp.tile([P, MC, 512], bf16, tag=f"qT{g}", name=f"qT{g}")
                    for g in range(QC)]
            if needs_bqk:
                bq_sb = pp.tile([P, MC], f32, tag="bq")
                bk_sb = pp.tile([P, MC], f32, tag="bk")
            if needs_bv:
                bv_sb = pp.tile([1, MC * P], bf16, tag="bv")
                ones1 = pp.tile([1, P], bf16, tag="ones1")

            eps_sb = pp.tile([P, 1], f32, tag="eps")
            nc.vector.memset(eps_sb[:], EPS)
            if needs_bv:
                nc.sync.dma_start(bv_sb[:], bvd)
                nc.vector.memset(ones1[:], 1.0)
            ident = pp.tile([P, P], bf16, tag="ident")
            make_identity(nc, ident[:])

            def ln_tile(src, t, dstT, dsti, apply_eng):
                """LN token tile t of src into dstT[:, dsti] ([P, EC, P])."""
                xt = xpool.tile([P, E], f32, tag="x")
                nc.sync.dma_start(xt[:], src[t * P:(t + 1) * P, :])
                stats = scr.tile([P, 6], f32, tag="bnstats")
                mv = scr.tile([P, 2], f32, tag="bnmv")
                nc.vector.bn_stats(stats[:], xt[:])
                nc.vector.bn_aggr(mv[:], stats[:])
                sig = scr.tile([P, 1], f32, tag="sig")
                nc.scalar.activation(
                    sig[:], mv[:, 1:2],
                    mybir.ActivationFunctionType.Sqrt, bias=eps_sb[:])
                rsig = scr.tile([P, 1], f32, tag="rsig")
                nc.vector.reciprocal(rsig[:], sig[:])
                xln = xlnp.tile([P, E], bf16, tag="xln")
                if apply_eng == "act":
                    # x*rsig + (-mu*rsig) on the scalar engine (idle during
                    # the prologue); DVE only computes the tiny bias term.
                    nmr = scr.tile([P, 1], f32, tag="nmr")
                    nc.vector.tensor_scalar(
                        nmr[:], mv[:, 0:1], -1.0, None, mybir.AluOpType.mult)
                    nc.vector.tensor_tensor(
                        nmr[:], nmr[:], rsig[:], mybir.AluOpType.mult)
                    nc.scalar.activation(
                        xln[:], xt[:], mybir.ActivationFunctionType.Identity,
                        bias=nmr[:], scale=rsig[:])
                elif apply_eng == "pool":
                    nc.gpsimd.tensor_scalar(
                        xln[:], xt[:], mv[:, 0:1], rsig[:],
                        mybir.AluOpType.subtract, mybir.AluOpType.mult)
                else:
                    nc.vector.tensor_scalar(
                        xln[:], xt[:], mv[:, 0:1], rsig[:],
                        mybir.AluOpType.subtract, mybir.AluOpType.mult)
                tp = psT.tile([P, E], bf16, tag="tp")
                for c in range(EC):
                    nc.tensor.transpose(
                        tp[:, c * P:(c + 1) * P], xln[:, c * P:(c + 1) * P],
                        ident[:])
                nc.vector.tensor_copy(
                    dstT[:, dsti], tp[:].rearrange("p (c n) -> p c n", n=P))

            def psum_to_sbuf(dst, src, eng, bias=None):
                """Copy a PSUM matmul result to SBUF (GPSIMD has no PSUM
                access, so this is ACT in the prologue / DVE elsewhere)."""
                if bias is not None:
                    nc.scalar.activation(
                        dst, src, mybir.ActivationFunctionType.Identity,
                        bias=bias)
                elif eng == "act":
                    nc.scalar.activation(
                        dst, src, mybir.ActivationFunctionType.Identity)
                else:
                    nc.vector.tensor_copy(dst, src)

            def kproj_group(i):
                for mc in range(MC):
                    ps = psS.tile([P, 3, 512], f32, tag="sp")
                    for c in range(EC):
                        nc.tensor.matmul(
                            ps[:, 0, 0:3 * P],
                            lhsT=wk_sb[:, c, mc * P:(mc + 1) * P],
                            rhs=xlnkvT_g[i][:, :, c, :],
                            start=(c == 0), stop=(c == EC - 1))
                    psum_to_sbuf(
                        kT_g[i][:, mc, :], ps[:, 0, 0:3 * P], "act",
                        bias=bk_sb[:, mc:mc + 1] if needs_bqk else None)

            def vproj_group(i):
                for ti in range(3):
                    ps = psS.tile([P, 3, 512], f32, tag="sp")
                    for c in range(EC):
                        nc.tensor.matmul(
                            ps[:, 0, 0:HC * D],
                            lhsT=xlnkvT_g[i][:, ti, c, :],
                            rhs=wv_sb[:, c, :],
                            start=(c == 0),
                            stop=(c == EC - 1 and not needs_bv))
                    if needs_bv:
                        nc.tensor.matmul(
                            ps[:, 0, 0:HC * D], lhsT=ones1[:], rhs=bv_sb[:],
                            start=False, stop=True)
                    vd = v_g[i][:, ti].rearrange("p (h d) -> p h d", d=D + 1)
                    psum_to_sbuf(
                        vd[:, :, 0:D],
                        ps[:, 0, 0:HC * D].rearrange("p (h d) -> p h d", d=D),
                        "act")
                    nc.gpsimd.memset(vd[:, :, D], 1.0)

            def qproj_group(g, eng):
                for mc in range(MC):
                    ps = psS.tile([P, 3, 512], f32, tag="sp")
                    for c in range(EC):
                        nc.tensor.matmul(
                            ps[:, 0, :],
                            lhsT=wq_sb[:, c, mc * P:(mc + 1) * P],
                            rhs=xlnqT_g[g][:, :, c, :],
                            start=(c == 0), stop=(c == EC - 1))
                    psum_to_sbuf(
                        qT_g[g][:, mc, :], ps[:, 0, :], eng,
                        bias=bq_sb[:, mc:mc + 1] if needs_bqk else None)

            # ---- prologue: kv side, then q group 0 ----
            for i in range(nkg):
                for ti in range(3):
                    ln_tile(xkv, 3 * i + ti, xlnkvT_g[i], ti, "act")
                    if i == 0 and ti == 0:
                        nc.sync.dma_start(
                            wk_sb[:], wk.rearrange("(c p) n -> p c n", p=P))
                        nc.sync.dma_start(
                            wv_sb[:], wv.rearrange("(c p) n -> p c n", p=P))
                        nc.sync.dma_start(
                            wq_sb[:], wq.rearrange("(c p) n -> p c n", p=P))
                        nc.sync.dma_start(
                            wo_sb[:], wo.rearrange("(c p) n -> p c n", p=P))
                        if needs_bqk:
                            nc.sync.dma_start(bq_sb[:], bqd)
                            nc.sync.dma_start(bk_sb[:], bkd)
                kproj_group(i)
                vproj_group(i)
                for j in range(3):
                    nc.sync.dma_start(mt_g[i][:, j, :], mtr[:, 3 * i + j, :])
            for ti in range(4):
                ln_tile(xq, ti, xlnqT_g[0], ti, "dve")
            qproj_group(0, "act")

            # ---- attention: 16 blocks of (q group g, head h) ----
            av_t = {}
            aT_t = {}

            def out_tile(g, qs):
                t = g * 4 + qs
                ps = psS.tile([P, 3, 512], f32, tag="sp")
                for mc in range(MC):
                    nc.tensor.matmul(
                        ps[:, 0, :],
                        lhsT=aT_t[g][:, mc, qs * P:(qs + 1) * P],
                        rhs=wo_sb[:, mc, :],
                        start=(mc == 0), stop=(mc == MC - 1))
                ob = outp.tile([P, E], bf16, tag="ob")
                nc.vector.tensor_copy(ob[:], ps[:, 0, :])
                nc.sync.dma_start(outd[t * P:(t + 1) * P, :], ob[:])

            for g in range(QC):
                av = avp.tile([P, 4, HC, D], bf16, tag="av", name=f"av{g}")
                av_t[g] = av
                aT_t[g] = aTp.tile([P, MC, 512], bf16, tag="aT",
                                   name=f"aT{g}")
                for h in range(HC):
                    mc = h // 2
                    po = (h % 2) * D
                    acc = psA.tile([P, 4, D + 1], f32, tag="acc")
                    pT = pTp.tile([P, nkt, 512], bf16, tag="pt")
                    # GPSIMD masks chunk 0 (its latency is covered by the
                    # later DVE-masked chunks: accumulation takes chunk 0
                    # last); DVE masks the rest.
                    for i in range(nkg):
                        sp = psS.tile([P, 3, 512], f32, tag="sp")
                        for j in range(3):
                            nc.tensor.matmul(
                                sp[:, j, :],
                                lhsT=kT_g[i][po:po + D, mc,
                                             j * P:(j + 1) * P],
                                rhs=qT_g[g][po:po + D, mc, :],
                                start=True, stop=True)
                        # q-side prep for group g+1 rides in the exp bubble
                        if i == 1 and g < QC - 1:
                            ln_tile(xq, (g + 1) * 4 + h, xlnqT_g[g + 1], h,
                                    "pool")
                        nc.scalar.activation(
                            pT[:, 3 * i:3 * i + 3, :], sp[:],
                            mybir.ActivationFunctionType.Exp, scale=SCALE)
                        eng = nc.gpsimd if i == 0 else nc.vector
                        eng.tensor_tensor(
                            pT[:, 3 * i:3 * i + 3, :],
                            pT[:, 3 * i:3 * i + 3, :],
                            mt_g[i][:, :, g * 512:(g + 1) * 512],
                            mybir.AluOpType.mult)
                        # acc holds 4 interleaved accumulation chains in one
                        # PSUM bank: start=True (whole-bank lazy zero) only on
                        # the first matmul; each chain's first write then
                        # overwrites via the pending-zero bytes; stop on the
                        # last matmul of the bank.
                        if i > 0:
                            for j in range(3):
                                kc = 3 * i + j
                                for qs in range(4):
                                    nc.tensor.matmul(
                                        acc[:, qs, :],
                                        lhsT=pT[:, kc, qs * P:(qs + 1) * P],
                                        rhs=v_g[i][:, j, h * (D + 1):
                                                   (h + 1) * (D + 1)],
                                        start=(i == 1 and j == 0
                                               and qs == 0),
                                        stop=False, skip_group_check=True)
                    for j in range(3):
                        for qs in range(4):
                            nc.tensor.matmul(
                                acc[:, qs, :],
                                lhsT=pT[:, j, qs * P:(qs + 1) * P],
                                rhs=v_g[0][:, j,
                                           h * (D + 1):(h + 1) * (D + 1)],
                                start=(nkg == 1 and j == 0 and qs == 0),
                                stop=(j == 2 and qs == 3),
                                skip_group_check=True)
                    # normalize: per-partition denominator in acc[:, :, 64]
                    dn = scr.tile([P, 4, 1], f32, tag="dn")
                    nc.vector.tensor_scalar(
                        dn[:, :, 0], acc[:, :, D], 1e-30, None,
                        mybir.AluOpType.add)
                    rcp = scr.tile([P, 4, 1], f32, tag="rcp")
                    nc.vector.reciprocal(rcp[:, :, 0], dn[:, :, 0])
                    nc.vector.tensor_tensor(
                        av[:, :, h, :], acc[:, :, 0:D],
                        rcp[:].to_broadcast((P, 4, D)), mybir.AluOpType.mult)
                    if h % 2 == 1:
                        # head pair mc complete: transpose into the bf16
                        # PSUM staging tile, one wide copy out to aT
                        pr = h // 2
                        tp = psT.tile([P, E], bf16, tag="tp")
                        for qs in range(4):
                            nc.tensor.transpose(
                                tp[:, qs * P:(qs + 1) * P],
                                av[:, qs, 2 * pr:2 * pr + 2, :], ident[:])
                        nc.vector.tensor_copy(aT_t[g][:, pr, :], tp[:])
                    if g > 0:
                        out_tile(g - 1, h)
                if g < QC - 1:
                    qproj_group(g + 1, "dve")
            for qs in range(4):
                out_tile(QC - 1, qs)

    # NOTE: _split_sync_waits is applied lazily in kernel() — the walrus
    # compile needs it, but CoreSim's race detector can't model the NoOps.
    return nc


def _get_nc(needs_bv: bool = False, reps: int = 1, nkt: int | None = None,
            needs_bqk: bool | None = None):
    global _LAST_KEY
    if nkt is None:
        if _LAST_KEY is not None:
            return _CACHE[_LAST_KEY]
        nkt = 9
    if needs_bqk is None:
        needs_bqk = needs_bv
    key = ("nc", nkt, needs_bqk, needs_bv)
    if key not in _CACHE:
        _CACHE[key] = _build(nkt, needs_bqk, needs_bv)
    _LAST_KEY = key
    return _CACHE[key]


def kernel(query, key_value, kv_mask, sparse_mask,
           ln_q_g, ln_q_b, ln_kv_g, ln_kv_b,
           Wq, bq, Wk, bk, Wv, bv, Wo, bo):
    query = np.asarray(query, np.float32)
    key_value = np.asarray(key_value, np.float32)
    kv_mask = np.asarray(kv_mask)
    sparse_mask = np.asarray(sparse_mask)
    B = query.shape[0]

    # Fold LN gain/bias into the projection weights (exact algebra):
    # (x_ln*g + b) @ W + c  ==  x_ln @ (g[:,None]*W) + (b@W + c)
    Wq_g = np.asarray(ln_q_g, np.float32)[:, None] * np.asarray(Wq, np.float32)
    Wk_g = np.asarray(ln_kv_g, np.float32)[:, None] * np.asarray(Wk, np.float32)
    Wv_g = np.asarray(ln_kv_g, np.float32)[:, None] * np.asarray(Wv, np.float32)
    bq_e = np.asarray(ln_q_b, np.float32) @ np.asarray(Wq, np.float32) + bq
    bk_e = np.asarray(ln_kv_b, np.float32) @ np.asarray(Wk, np.float32) + bk
    bv_e = np.asarray(ln_kv_b, np.float32) @ np.asarray(Wv, np.float32) + bv

    needs_bqk = bool(np.any(bq_e != 0.0) or np.any(bk_e != 0.0))
    needs_bv = bool(np.any(bv_e != 0.0))

    # Compact the kv sequence: tokens with kv_mask=0 are masked for every
    # query, so drop them and pad to a multiple of 384 (3 kv tiles).
    valid = [np.flatnonzero(kv_mask[b]) for b in range(B)]
    nv_max = max(1, max(len(v) for v in valid))
    nkt = 3 * math.ceil(math.ceil(nv_max / P) / 3)
    KT = nkt * P

    nc = _get_nc(needs_bv, nkt=nkt, needs_bqk=needs_bqk)

    xkvc = np.zeros((B, KT, E), np.float32)
    mtc = np.zeros((B, KT, T), BF16)
    for b in range(B):
        nv = len(valid[b])
        xkvc[b, :nv] = key_value[b][valid[b]]
        mtc[b, :nv] = sparse_mask[b].T[valid[b]].astype(BF16)

    in_maps = []
    for c in range(8):
        b, hg = c // 2, c % 2
        hs = slice(hg * MC * P, (hg + 1) * MC * P)
        m = {
            "xq": np.ascontiguousarray(query[b]),
            "xkv": np.ascontiguousarray(xkvc[b]),
            "wq": np.ascontiguousarray(Wq_g[:, hs]).astype(BF16),
            "wk": np.ascontiguousarray(Wk_g[:, hs]).astype(BF16),
            "wv": np.ascontiguousarray(Wv_g[:, hs]).astype(BF16),
            "wo": np.ascontiguousarray(
                np.asarray(Wo, np.float32)[hs, :]).astype(BF16),
            "mt": np.ascontiguousarray(mtc[b]),
        }
        if needs_bqk:
            m["bq"] = np.ascontiguousarray(bq_e[hs].reshape(MC, P).T)
            m["bk"] = np.ascontiguousarray(bk_e[hs].reshape(MC, P).T)
        if needs_bv:
            m["bv"] = bv_e[hs].astype(BF16).reshape(1, MC * P)
        in_maps.append(m)

    if not getattr(nc, "_sync_waits_split", False):
        _split_sync_waits(nc)
        nc._sync_waits_split = True
    res = bass_utils.run_bass_kernel_spmd(
        nc, in_maps, core_ids=list(range(8)),
        trace=bool(os.environ.get("KERNEL_TRACE")))
    globals()["LAST_RESULTS"] = res

    bo_f = np.asarray(bo, np.float32)
    out = np.empty((B, T, E), np.float32)
    for b in range(B):
        out[b] = (res.results[2 * b]["out"].astype(np.float32)
                  + res.results[2 * b + 1]["out"].astype(np.float32) + bo_f)
    return out


# revision 18
# speedup vs baseline: 1.7488x; 1.1989x over previous
"""Trainium2 Bass kernel for nn_CrossAttentionLayer (sparse cross attention).

Sharding: 8 cores = 4 batches x 2 head-groups. Core c handles batch c//2 and
heads [4*(c%2), 4*(c%2)+4). Host compacts the kv sequence per batch using
kv_mask (a kv token masked out by kv_mask is invisible to every query), padding
to a multiple of 384 tokens; the padded sparse-mask columns are zero so padding
contributes nothing. Each core computes LN + q/k/v projections for its shard,
masked attention, and a partial out-projection. Host sums the two per-batch
partials and adds bo.

Device algorithm (per core), matmuls bf16 with fp32 PSUM accumulation:
  xlnT   = transpose(layernorm(x))          (LN gains/biases folded into W/b)
  qT/kT  = W.T @ xlnT   [d, tok]
  v      = xlnT.T @ Wv  [tok, d] with a ones column per head (softmax denom)
  per (q-block of 512, head):
    sT   = kT-chunks.T @ qT               [k, q] scores, 3-kv-tile chunks
    pT   = exp(sT * scale) * sparse_mask  (ACT exp; mask multiply split
                                           DVE/GPSIMD, GPSIMD chunk last)
    acc  = pT-chunks.T @ [v | 1]          [q, 65] PSUM accumulation over k
    av   = acc[:, 0:64] / acc[:, 64]      (per-partition normalize on DVE)
    aT   = transpose(av)                  (PE transpose, bf16 PSUM staging)
  out    = aT-chunks @ Wo                 [q, E] partial, bf16 to HBM
"""

import math
import os

import numpy as np
import ml_dtypes

import bass_rust
import concourse.bass as bass
import concourse.mybir as mybir
import concourse.tile as tile
from concourse import bass_utils
from concourse.masks import make_identity
from concourse.vector_clock import ScopedClock


class _TileContext(tile.TileContext):
    """TileContext whose kernel-tail drain is split into single-wait drains.

    The walrus build in this environment rejects >1 sync-wait on a Drain
    (CTRL_NO struct): "Too many sync wait commands". The stock
    _drain_and_barrier attaches one wait per outstanding semaphore to a
    single Drain; emit one Drain per wait instead.
    """

    def _drain_and_barrier(self, tick_clock, wait_clock):
        drain_inst = self.nc.sync.drain()
        wait_clock.add_sem_waits(
            drain_inst.ins, ScopedClock({None: tick_clock.global_clock})
        )
        si = drain_inst.ins.sync_info
        if si is not None and si.on_wait and len(si.on_wait) > 1:
            waits = list(si.on_wait)
            drain_inst.ins.sync_info = bass_rust.SyncInfo(
                on_wait=[waits[0]], on_update=si.on_update or [])
            for w in waits[1:]:
                extra = self.nc.sync.drain()
                extra.ins.sync_info = bass_rust.SyncInfo(
                    on_wait=[w], on_update=[])

        self.nc.all_engine_barrier()
        assert self.sems is not None
        popped = self.nc._tile_sem_poison_stack.pop()
        assert popped is self._sem_poison
        self.nc.clear_and_free_semaphores(list(self.sems.allocated().values()))
        self.nc.all_engine_barrier()


def _split_sync_waits(nc):
    """Cap every instruction at one sync wait.

    This walrus build rejects instructions carrying more than one sem wait
    ("Too many sync wait commands", setupSyncWait) across several structs
    (Drain, DMACopy, ...). Move excess waits onto no-op instructions placed
    immediately before the offender on the same engine — identical ordering
    semantics, one wait per instruction.
    """
    for f in nc.m.functions:
        for bb in f.blocks:
            insns = bb.instructions
            out = []
            changed = False
            for ins in insns:
                si = ins.sync_info
                if si is not None and si.on_wait and len(si.on_wait) > 1:
                    waits = list(si.on_wait)
                    for w in waits[:-1]:
                        nop = mybir.InstNoOp(
                            name=nc.get_next_instruction_name(),
                            engine=ins.engine,
                            ins=[], outs=[],
                            sync_info=bass_rust.SyncInfo(
                                on_wait=[w], on_update=[]),
                        )
                        out.append(nop)
                    ins.sync_info = bass_rust.SyncInfo(
                        on_wait=[waits[-1]], on_update=si.on_update or [])
                    changed = True
                out.append(ins)
            if changed:
                bb.instructions = out


BF16 = ml_dtypes.bfloat16

E = 512
H = 8
D = 64
T = 2048           # query tokens
P = 128
NQT = T // P       # 16 query token tiles
EC = E // P        # 4 contraction chunks
HC = 4             # heads per core
MC = 2             # 128-wide chunks of this core's 256 head dims
QC = 4             # 512-wide query chunks (= query tile groups of 4)
SCALE = float(D) ** -0.5
EPS = 1e-5

_CACHE = {}
_LAST_KEY = None


def _build(nkt: int, needs_bqk: bool, needs_bv: bool):
    assert nkt % 3 == 0, "kv tiles padded to a multiple of 3"
    nkg = nkt // 3               # kv groups of 3 tiles / score chunks of 3
    KT = nkt * P

    nc = bass.Bass("TRN2", target_bir_lowering=False, debug=False,
                   num_devices=8)
    f32 = mybir.dt.float32
    bf16 = mybir.dt.bfloat16

    xq = nc.dram_tensor("xq", [T, E], bf16, kind="ExternalInput").ap()
    xkv = nc.dram_tensor("xkv", [KT, E], bf16, kind="ExternalInput").ap()
    wq = nc.dram_tensor("wq", [E, MC * P], bf16, kind="ExternalInput").ap()
    wk = nc.dram_tensor("wk", [E, MC * P], bf16, kind="ExternalInput").ap()
    wv = nc.dram_tensor("wv", [E, MC * P], bf16, kind="ExternalInput").ap()
    wo = nc.dram_tensor("wo", [MC * P, E], bf16, kind="ExternalInput").ap()
    mtd = nc.dram_tensor("mt", [KT, T], bf16, kind="ExternalInput").ap()
    if needs_bqk:
        bqd = nc.dram_tensor("bq", [P, MC], f32, kind="ExternalInput").ap()
        bkd = nc.dram_tensor("bk", [P, MC], f32, kind="ExternalInput").ap()
    if needs_bv:
        bvd = nc.dram_tensor("bv", [1, MC * P], bf16, kind="ExternalInput").ap()
    outd = nc.dram_tensor("out", [T, E], bf16, kind="ExternalOutput").ap()

    mtr = mtd.rearrange("(c p) q -> p c q", p=P)

    with _TileContext(nc) as tc:
        with (
            tc.tile_pool(name="persist", bufs=1) as pp,
            tc.tile_pool(name="xs", bufs=4) as xpool,
            tc.tile_pool(name="xpre", bufs=9) as xpre,
            tc.tile_pool(name="xln", bufs=3) as xlnp,
            tc.tile_pool(name="scratch", bufs=4) as scr,
            tc.tile_pool(name="ostage", bufs=2) as outp,
            tc.tile_pool(name="pt", bufs=2) as pTp,
            tc.tile_pool(name="av", bufs=2) as avp,
            tc.tile_pool(name="at", bufs=2) as aTp,
            tc.tile_pool(name="psS", bufs=2, space="PSUM") as psS,
            tc.tile_pool(name="psA", bufs=1, space="PSUM") as psA,
            tc.tile_pool(name="psO", bufs=1, space="PSUM") as psO,
        ):
            # ---- persistent SBUF tensors ----
            wq_sb = pp.tile([P, EC, MC * P], bf16, tag="wq")
            wk_sb = pp.tile([P, EC, MC * P], bf16, tag="wk")
            wv_sb = pp.tile([P, EC, MC * P], bf16, tag="wv")
            wo_sb = pp.tile([P, MC, E], bf16, tag="wo")
            mt_g = [pp.tile([P, 3, T], bf16, tag=f"mt{i}", name=f"mt{i}")
                    for i in range(nkg)]
            xlnkvT_g = [pp.tile([P, 3, EC, P], bf16, tag=f"xlnkv{i}",
                                name=f"xlnkv{i}") for i in range(nkg)]
            xlnqT_g = [pp.tile([P, 4, EC, P], bf16, tag=f"xlnq{g}",
                               name=f"xlnq{g}") for g in range(QC)]
            kT_g = [pp.tile([P, MC, 3 * P], bf16, tag=f"kT{i}",
                            name=f"kT{i}") for i in range(nkg)]
            v_g = [pp.tile([P, 3, HC * (D + 1)], bf16, tag=f"v{i}",
                           name=f"v{i}") for i in range(nkg)]
            qT_g = [pp.tile([P, MC, 512], bf16, tag=f"qT{g}", name=f"qT{g}")
                    for g in range(QC)]
            if needs_bqk:
                bq_sb = pp.tile([P, MC], f32, tag="bq")
                bk_sb = pp.tile([P, MC], f32, tag="bk")
            if needs_bv:
                bv_sb = pp.tile([1, MC * P], bf16, tag="bv")
                ones1 = pp.tile([1, P], bf16, tag="ones1")

            eps_sb = pp.tile([P, 1], f32, tag="eps")
            nc.vector.memset(eps_sb[:], EPS)
            if needs_bv:
                nc.sync.dma_start(bv_sb[:], bvd)
                nc.vector.memset(ones1[:], 1.0)
            ident = pp.tile([P, P], bf16, tag="ident")
            make_identity(nc, ident[:])

            def ln_s1(src, t):
                """LN stage 1: load + stats + sqrt (DVE/ACT)."""
                xt = xpool.tile([P, E], bf16, tag="x")
                nc.sync.dma_start(xt[:], src[t * P:(t + 1) * P, :])
                stats = scr.tile([P, 6], f32, tag="bnstats")
                mv = scr.tile([P, 2], f32, tag="bnmv")
                nc.vector.bn_stats(stats[:], xt[:])
                nc.vector.bn_aggr(mv[:], stats[:])
                sig = scr.tile([P, 1], f32, tag="sig")
                nc.scalar.activation(
                    sig[:], mv[:, 1:2],
                    mybir.ActivationFunctionType.Sqrt, bias=eps_sb[:])
                return xt, mv, sig

            def ln_s2(st, dstT, dsti, apply_eng, copy_eng):
                """LN stage 2: normalize + transpose + copy out."""
                xt, mv, sig = st
                rsig = scr.tile([P, 1], f32, tag="rsig")
                nc.vector.reciprocal(rsig[:], sig[:])
                xln = xlnp.tile([P, E], bf16, tag="xln")
                if apply_eng == "act":
                    nmr = scr.tile([P, 1], f32, tag="nmr")
                    nc.vector.tensor_scalar(
                        nmr[:], mv[:, 0:1], -1.0, None, mybir.AluOpType.mult)
                    nc.vector.tensor_tensor(
                        nmr[:], nmr[:], rsig[:], mybir.AluOpType.mult)
                    nc.scalar.activation(
                        xln[:], xt[:], mybir.ActivationFunctionType.Identity,
                        bias=nmr[:], scale=rsig[:])
                elif apply_eng == "pool":
                    nc.gpsimd.tensor_scalar(
                        xln[:], xt[:], mv[:, 0:1], rsig[:],
                        mybir.AluOpType.subtract, mybir.AluOpType.mult)
                else:
                    nc.vector.tensor_scalar(
                        xln[:], xt[:], mv[:, 0:1], rsig[:],
                        mybir.AluOpType.subtract, mybir.AluOpType.mult)
                tps = psA.tile([P, 4, D + 1], f32, tag="acc")
                tp = tps[:].rearrange("p a b -> p (a b)").bitcast(bf16)
                for c in range(EC):
                    nc.tensor.transpose(
                        tp[:, c * P:(c + 1) * P], xln[:, c * P:(c + 1) * P],
                        ident[:])
                if copy_eng == "act":
                    nc.scalar.activation(
                        dstT[:, dsti],
                        tp[:, 0:E].rearrange("p (c n) -> p c n", n=P),
                        mybir.ActivationFunctionType.Identity)
                else:
                    nc.vector.tensor_copy(
                        dstT[:, dsti],
                        tp[:, 0:E].rearrange("p (c n) -> p c n", n=P))

            def ln_tile(src, t, dstT, dsti, apply_eng, xt=None):
                """LN token tile t of src into dstT[:, dsti] ([P, EC, P])."""
                if xt is None:
                    xt = xpool.tile([P, E], bf16, tag="x")
                    nc.sync.dma_start(xt[:], src[t * P:(t + 1) * P, :])
                stats = scr.tile([P, 6], f32, tag="bnstats")
                mv = scr.tile([P, 2], f32, tag="bnmv")
                nc.vector.bn_stats(stats[:], xt[:])
                nc.vector.bn_aggr(mv[:], stats[:])
                sig = scr.tile([P, 1], f32, tag="sig")
                nc.scalar.activation(
                    sig[:], mv[:, 1:2],
                    mybir.ActivationFunctionType.Sqrt, bias=eps_sb[:])
                rsig = scr.tile([P, 1], f32, tag="rsig")
                nc.vector.reciprocal(rsig[:], sig[:])
                xln = xlnp.tile([P, E], bf16, tag="xln")
                if apply_eng == "act":
                    # x*rsig + (-mu*rsig) on the scalar engine (idle during
                    # the prologue); DVE only computes the tiny bias term.
                    nmr = scr.tile([P, 1], f32, tag="nmr")
                    nc.vector.tensor_scalar(
                        nmr[:], mv[:, 0:1], -1.0, None, mybir.AluOpType.mult)
                    nc.vector.tensor_tensor(
                        nmr[:], nmr[:], rsig[:], mybir.AluOpType.mult)
                    nc.scalar.activation(
                        xln[:], xt[:], mybir.ActivationFunctionType.Identity,
                        bias=nmr[:], scale=rsig[:])
                elif apply_eng == "pool":
                    nc.gpsimd.tensor_scalar(
                        xln[:], xt[:], mv[:, 0:1], rsig[:],
                        mybir.AluOpType.subtract, mybir.AluOpType.mult)
                else:
                    nc.vector.tensor_scalar(
                        xln[:], xt[:], mv[:, 0:1], rsig[:],
                        mybir.AluOpType.subtract, mybir.AluOpType.mult)
                nc.sync.dma_start_transpose(dstT[:, dsti], xln[:])

            def psum_to_sbuf(dst, src, eng, bias=None):
                if bias is not None:
                    nc.scalar.activation(
                        dst, src, mybir.ActivationFunctionType.Identity,
                        bias=bias)
                elif eng == "act":
                    nc.scalar.activation(
                        dst, src, mybir.ActivationFunctionType.Identity)
                elif eng == "pool":
                    nc.gpsimd.tensor_copy(dst, src)
                else:
                    nc.vector.tensor_copy(dst, src)

            def kproj_group(i):
                for mc in range(MC):
                    ps = psS.tile([P, 3, 512], f32, tag="sp")
                    for c in range(EC):
                        nc.tensor.matmul(
                            ps[:, 0, 0:3 * P],
                            lhsT=wk_sb[:, c, mc * P:(mc + 1) * P],
                            rhs=xlnkvT_g[i][:, :, c, :],
                            start=(c == 0), stop=(c == EC - 1))
                    psum_to_sbuf(
                        kT_g[i][:, mc, :], ps[:, 0, 0:3 * P], "act",
                        bias=bk_sb[:, mc:mc + 1] if needs_bqk else None)

            def vproj_group(i):
                for ti in range(3):
                    ps = psS.tile([P, 3, 512], f32, tag="sp")
                    for c in range(EC):
                        nc.tensor.matmul(
                            ps[:, 0, 0:HC * D],
                            lhsT=xlnkvT_g[i][:, ti, c, :],
                            rhs=wv_sb[:, c, :],
                            start=(c == 0),
                            stop=(c == EC - 1 and not needs_bv))
                    if needs_bv:
                        nc.tensor.matmul(
                            ps[:, 0, 0:HC * D], lhsT=ones1[:], rhs=bv_sb[:],
                            start=False, stop=True)
                    vd = v_g[i][:, ti].rearrange("p (h d) -> p h d", d=D + 1)
                    psum_to_sbuf(
                        vd[:, :, 0:D],
                        ps[:, 0, 0:HC * D].rearrange("p (h d) -> p h d", d=D),
                        "act")
                    nc.gpsimd.memset(vd[:, :, D], 1.0)

            def qproj_group(g, eng):
                for mc in range(MC):
                    ps = psS.tile([P, 3, 512], f32, tag="sp")
                    for c in range(EC):
                        nc.tensor.matmul(
                            ps[:, 0, :],
                            lhsT=wq_sb[:, c, mc * P:(mc + 1) * P],
                            rhs=xlnqT_g[g][:, :, c, :],
                            start=(c == 0), stop=(c == EC - 1))
                    psum_to_sbuf(
                        qT_g[g][:, mc, :], ps[:, 0, :], eng,
                        bias=bq_sb[:, mc:mc + 1] if needs_bqk else None)

            # ---- prologue: kv side then q group 0, 2-stage pipelined
            # so DVE's stage-1 stats of tile t+1 run while tile t crosses
            # ACT/PE for normalize+transpose.
            seq = ([(xkv, t, xlnkvT_g[t // 3], t % 3, "act", "dve")
                    for t in range(3 * nkg)]
                   + [(xq, t, xlnqT_g[0], t, "dve", "act")
                      for t in range(4)]
                   + [(xq, 4 + t, xlnqT_g[1], t, "pool", "dve")
                      for t in range(3)])
            st = {}
            for idx in range(len(seq) + 1):
                if idx < len(seq):
                    st[idx] = ln_s1(seq[idx][0], seq[idx][1])
                    if idx == 0:
                        nc.sync.dma_start(
                            wk_sb[:], wk.rearrange("(c p) n -> p c n", p=P))
                        nc.sync.dma_start(
                            wv_sb[:], wv.rearrange("(c p) n -> p c n", p=P))
                        nc.sync.dma_start(
                            wq_sb[:], wq.rearrange("(c p) n -> p c n", p=P))
                        nc.sync.dma_start(
                            wo_sb[:], wo.rearrange("(c p) n -> p c n", p=P))
                        if needs_bqk:
                            nc.sync.dma_start(bq_sb[:], bqd)
                            nc.sync.dma_start(bk_sb[:], bkd)
                if idx > 0:
                    p = idx - 1
                    _, _, dstT, dsti, a_eng, c_eng = seq[p]
                    ln_s2(st.pop(p), dstT, dsti, a_eng, c_eng)
                    if p < 3 * nkg and p % 3 == 2:
                        kproj_group(p // 3)
                        vproj_group(p // 3)
            for i in range(nkg):
                for j in range(3):
                    nc.sync.dma_start(mt_g[i][:, j, :], mtr[:, 3 * i + j, :])
            qproj_group(0, "act")
            xpre_t = {}
            for t in range(7, 16):
                xpre_t[t] = xpre.tile([P, E], bf16, tag="xpre",
                                      name=f"xpre{t}")
                nc.sync.dma_start(xpre_t[t][:], xq[t * P:(t + 1) * P, :])

            # ---- attention: 16 blocks of (q group g, head h) ----
            # Software-pipelined: each block's chunk-0 (scores+exp+GPSIMD
            # mask) is emitted during the PREVIOUS block, so the scalar
            # engine's exp stream never waits for the slow GPSIMD mask
            # (chunk 0 is accumulated last).
            av_t = {}
            aT_t = {}
            pT_t = {}
            blocks = [(g, h) for g in range(QC) for h in range(HC)]

            def out_tile(g, qs):
                t = g * 4 + qs
                ps = psO.tile([P, E], f32, tag="po")
                for mc in range(MC):
                    nc.tensor.matmul(
                        ps[:],
                        lhsT=aT_t[g][:, mc, qs * P:(qs + 1) * P],
                        rhs=wo_sb[:, mc, :],
                        start=(mc == 0), stop=(mc == MC - 1))
                ob = outp.tile([P, E], bf16, tag="ob")
                nc.vector.tensor_copy(ob[:], ps[:])
                nc.sync.dma_start(outd[t * P:(t + 1) * P, :], ob[:])

            def score_chunk(bi, i):
                """Scores + exp + mask for chunk i of block bi."""
                g, h = blocks[bi]
                mc = h // 2
                po = (h % 2) * D
                pT = pT_t[bi]
                sp = psS.tile([P, 3, 512], f32, tag="sp")
                for j in range(3):
                    nc.tensor.matmul(
                        sp[:, j, :],
                        lhsT=kT_g[i][po:po + D, mc, j * P:(j + 1) * P],
                        rhs=qT_g[g][po:po + D, mc, :],
                        start=True, stop=True)
                nc.scalar.activation(
                    pT[:, 3 * i:3 * i + 3, :], sp[:],
                    mybir.ActivationFunctionType.Exp, scale=SCALE)
                if i == 0 and nkg > 1 and bi < len(blocks) - 1:
                    # GPSIMD masks 2 of chunk 0's kv tiles (its latency is
                    # covered: chunk 0 is accumulated last), DVE the third
                    nc.gpsimd.tensor_tensor(
                        pT[:, 0:2, :], pT[:, 0:2, :],
                        mt_g[0][:, 0:2, g * 512:(g + 1) * 512],
                        mybir.AluOpType.mult)
                    nc.vector.tensor_tensor(
                        pT[:, 2:3, :], pT[:, 2:3, :],
                        mt_g[0][:, 2:3, g * 512:(g + 1) * 512],
                        mybir.AluOpType.mult)
                else:
                    nc.vector.tensor_tensor(
                        pT[:, 3 * i:3 * i + 3, :],
                        pT[:, 3 * i:3 * i + 3, :],
                        mt_g[i][:, :, g * 512:(g + 1) * 512],
                        mybir.AluOpType.mult)

            def attnv_chunk(bi, i, acc, start, stop):
                g, h = blocks[bi]
                pT = pT_t[bi]
                for j in range(3):
                    kc = 3 * i + j
                    for qs in range(4):
                        nc.tensor.matmul(
                            acc[:, qs, :],
                            lhsT=pT[:, kc, qs * P:(qs + 1) * P],
                            rhs=v_g[i][:, j, h * (D + 1):(h + 1) * (D + 1)],
                            start=(start and j == 0 and qs == 0),
                            stop=(stop and j == 2 and qs == 3),
                            skip_group_check=True)

            # look-ahead work queue: one item per block, finishing group
            # g+1's LN pieces and projection one group early so chunk-0
            # preloads can cross group boundaries.
            thunks = []
            if QC > 1:
                thunks.append(("ln", 1, 3))
                thunks.append(("qp", 1, 0))
            for g2 in range(2, QC):
                for ti in range(4):
                    thunks.append(("ln", g2, ti))
                thunks.append(("qp", g2, 0))

            def run_thunk(bi):
                if bi >= len(thunks):
                    return
                kind, g2, ti = thunks[bi]
                if kind == "ln":
                    t = g2 * 4 + ti
                    ln_tile(xq, t, xlnqT_g[g2], ti, "pool",
                            xt=xpre_t[t])
                else:
                    qproj_group(g2, "dve")

            for bi, (g, h) in enumerate(blocks):
                mc = h // 2
                if h == 0:
                    av_t[g] = avp.tile([P, 4, HC, D], bf16, tag="av",
                                       name=f"av{g}")
                    aT_t[g] = aTp.tile([P, MC, 512], bf16, tag="aT",
                                       name=f"aT{g}")
                av = av_t[g]
                if bi not in pT_t:
                    pT_t[bi] = pTp.tile([P, nkt, 512], bf16, tag="pt",
                                        name=f"pT{bi}")
                    score_chunk(bi, 0)
                acc = psA.tile([P, 4, D + 1], f32, tag="acc")
                for i in range(1, nkg):
                    score_chunk(bi, i)
                run_thunk(bi)
                if bi + 1 < len(blocks):
                    # preload next block's chunk 0 so ACT stays fed while
                    # GPSIMD masks this block's chunk 0
                    pT_t[bi + 1] = pTp.tile([P, nkt, 512], bf16, tag="pt",
                                            name=f"pT{bi + 1}")
                    score_chunk(bi + 1, 0)
                for i in range(1, nkg):
                    attnv_chunk(bi, i, acc, start=(i == 1), stop=False)
                attnv_chunk(bi, 0, acc, start=(nkg == 1), stop=True)
                # normalize: per-partition denominator in acc[:, :, 64]
                dn = scr.tile([P, 4, 1], f32, tag="dn")
                nc.vector.tensor_scalar(
                    dn[:, :, 0], acc[:, :, D], 1e-30, None,
                    mybir.AluOpType.add)
                rcp = scr.tile([P, 4, 1], f32, tag="rcp")
                nc.vector.reciprocal(rcp[:, :, 0], dn[:, :, 0])
                nc.vector.tensor_tensor(
                    av[:, :, h, :], acc[:, :, 0:D],
                    rcp[:].to_broadcast((P, 4, D)), mybir.AluOpType.mult)
                if h % 2 == 1:
                    # head pair mc complete: transpose into acc's PSUM bank
                    # (free after the normalize read) via bitcast views,
                    # then one strided copy of all four q-tiles to aT
                    pr = h // 2
                    for qs in range(4):
                        nc.tensor.transpose(
                            acc[:, qs, 0:D].bitcast(bf16),
                            av[:, qs, 2 * pr:2 * pr + 2, :], ident[:])
                    nc.vector.tensor_copy(
                        aT_t[g][:, pr, :].rearrange("p (q n) -> p q n", n=P),
                        acc[:].bitcast(bf16)[:, :, 0:P])
                if g > 0:
                    out_tile(g - 1, h)
            for qs in range(4):
                out_tile(QC - 1, qs)

    # NOTE: _split_sync_waits is applied lazily in kernel() — the walrus
    # compile needs it, but CoreSim's race detector can't model the NoOps.
    return nc


def _get_nc(needs_bv: bool = False, reps: int = 1, nkt: int | None = None,
            needs_bqk: bool | None = None):
    global _LAST_KEY
    if nkt is None:
        if _LAST_KEY is not None:
            return _CACHE[_LAST_KEY]
        nkt = 9
    if needs_bqk is None:
        needs_bqk = needs_bv
    key = ("nc", nkt, needs_bqk, needs_bv)
    if key not in _CACHE:
        _CACHE[key] = _build(nkt, needs_bqk, needs_bv)
    _LAST_KEY = key
    return _CACHE[key]


def kernel(query, key_value, kv_mask, sparse_mask,
           ln_q_g, ln_q_b, ln_kv_g, ln_kv_b,
           Wq, bq, Wk, bk, Wv, bv, Wo, bo):
    query = np.asarray(query, np.float32)
    key_value = np.asarray(key_value, np.float32)
    kv_mask = np.asarray(kv_mask)
    sparse_mask = np.asarray(sparse_mask)
    B = query.shape[0]

    # Fold LN gain/bias into the projection weights (exact algebra):
    # (x_ln*g + b) @ W + c  ==  x_ln @ (g[:,None]*W) + (b@W + c)
    Wq_g = np.asarray(ln_q_g, np.float32)[:, None] * np.asarray(Wq, np.float32)
    Wk_g = np.asarray(ln_kv_g, np.float32)[:, None] * np.asarray(Wk, np.float32)
    Wv_g = np.asarray(ln_kv_g, np.float32)[:, None] * np.asarray(Wv, np.float32)
    bq_e = np.asarray(ln_q_b, np.float32) @ np.asarray(Wq, np.float32) + bq
    bk_e = np.asarray(ln_kv_b, np.float32) @ np.asarray(Wk, np.float32) + bk
    bv_e = np.asarray(ln_kv_b, np.float32) @ np.asarray(Wv, np.float32) + bv

    needs_bqk = bool(np.any(bq_e != 0.0) or np.any(bk_e != 0.0))
    needs_bv = bool(np.any(bv_e != 0.0))

    # Compact the kv sequence: tokens with kv_mask=0 are masked for every
    # query, so drop them and pad to a multiple of 384 (3 kv tiles).
    valid = [np.flatnonzero(kv_mask[b]) for b in range(B)]
    nv_max = max(1, max(len(v) for v in valid))
    nkt = 3 * math.ceil(math.ceil(nv_max / P) / 3)
    KT = nkt * P

    nc = _get_nc(needs_bv, nkt=nkt, needs_bqk=needs_bqk)

    xkvc = np.zeros((B, KT, E), np.float32)
    mtc = np.zeros((B, KT, T), BF16)
    for b in range(B):
        nv = len(valid[b])
        xkvc[b, :nv] = key_value[b][valid[b]]
        mtc[b, :nv] = sparse_mask[b].T[valid[b]].astype(BF16)

    in_maps = []
    for c in range(8):
        b, hg = c // 2, c % 2
        hs = slice(hg * MC * P, (hg + 1) * MC * P)
        m = {
            "xq": np.ascontiguousarray(query[b]).astype(BF16),
            "xkv": np.ascontiguousarray(xkvc[b]).astype(BF16),
            "wq": np.ascontiguousarray(Wq_g[:, hs]).astype(BF16),
            "wk": np.ascontiguousarray(Wk_g[:, hs]).astype(BF16),
            "wv": np.ascontiguousarray(Wv_g[:, hs]).astype(BF16),
            "wo": np.ascontiguousarray(
                np.asarray(Wo, np.float32)[hs, :]).astype(BF16),
            "mt": np.ascontiguousarray(mtc[b]),
        }
        if needs_bqk:
            m["bq"] = np.ascontiguousarray(bq_e[hs].reshape(MC, P).T)
            m["bk"] = np.ascontiguousarray(bk_e[hs].reshape(MC, P).T)
        if needs_bv:
            m["bv"] = bv_e[hs].astype(BF16).reshape(1, MC * P)
        in_maps.append(m)

    if not getattr(nc, "_sync_waits_split", False):
        _split_sync_waits(nc)
        nc._sync_waits_split = True
    res = bass_utils.run_bass_kernel_spmd(
        nc, in_maps, core_ids=list(range(8)),
        trace=bool(os.environ.get("KERNEL_TRACE")))
    globals()["LAST_RESULTS"] = res

    bo_f = np.asarray(bo, np.float32)
    out = np.empty((B, T, E), np.float32)
    for b in range(B):
        out[b] = (res.results[2 * b]["out"].astype(np.float32)
                  + res.results[2 * b + 1]["out"].astype(np.float32) + bo_f)
    return out
